# revision 27
# baseline (speedup 1.0000x reference)
"""GATv2 model kernel for Trainium2 (Bass/Tile), data-parallel over batch on 8 cores.

Model (per graph b): input MLP -> 4 GATv2 layers (dense N^2 attention with
edge features) -> sum-pool -> linear head.  B=16, N=128, HID=128, H=8, C=16.

Key structural fact: cat[i,j] takes only K*K+K = 20 distinct values (K=4
orbits of 32 consecutive nodes; diagonal i==j uses its own 4 categories).
So e_feat has 20 distinct rows and the whole e-transform collapses to a
host-side [20, HID] table.  Per-core layout (2 graphs per core):

  - h kept fp32 [node, hid]; hT/xlT/xrT produced per layer via PE transpose +
    fp16 copies; all weight matmuls run fp16 (1 cyc/row vs 4 for fp32).
  - u[hc, (q, i)] = xlT + e_offdiag(pos(i), q) built by one DVE op; message
    m for a 16-target block = one DVE STT (u + xr broadcast, fp16), then
    ScalarE Prelu(alpha=0.2) gives lrelu(m) -- scores are att.lrelu(m)
    computed exactly by one per-target fp16 matmul into its own 8-column
    PSUM range (no accumulation chain), then one exp -> fp16 scores.
  - diagonal (i==j category) fixed exactly post-hoc: correct/wrong diagonal
    scores computed per target in [j, h] layout (2 small matmuls + exp),
    delta = exp(sd)-exp(sw) patches the aggregation output po and the
    softmax normalizer column -- pointwise in j, no scatter.
  - aggregation via per-head matmuls with [xl | 1] (fp16) producing the
    unnormalized output and softmax normalizer Z in one PSUM tile.
  - LayerNorm rsqrt via exp(-0.5*ln(var+eps)); an ACT-table patch keeps all
    activations in the one table containing exp+ln (no per-LN table reloads).
"""

import numpy as np
from contextlib import ExitStack

import concourse.bacc as bacc
import concourse.bass as bass
import concourse.tile as tile
from concourse import mybir
from concourse.masks import make_identity

F32 = mybir.dt.float32
F16 = mybir.dt.float16
AF = mybir.ActivationFunctionType
OP = mybir.AluOpType
AX = mybir.AxisListType

B, N, HID, H, C, L, K = 16, 128, 128, 8, 16, 4, 4
NCORES = 8
BL = B // NCORES          # graphs per core
NEG = 0.2                 # leaky relu slope
EPS = 1e-5
AUG = 17                  # head dim + 1 (softmax normalizer column)
NO = N // K               # nodes per orbit (32)
JBB = 16                  # targets per message block
NBB = N // JBB

# LN vector slots in the replicated-params tile
LN1G, LN1B, LN2G, LN2B = 0, 1, 2, 3
LNG0, LNB0 = 4, 8


def _ln_free(nc, wp, sp, pin, g_ap, b_ap, out_ap, uid, zb, epsb):
    """LayerNorm along the free dim of pin [128, D] -> out_ap (SBUF)."""
    D = pin.shape[-1]
    mu = sp.tile([128, 1], F32, tag=f"mu{uid}")
    nc.vector.tensor_reduce(mu, pin, axis=AX.X, op=OP.add)
    nc.vector.tensor_scalar_mul(mu, mu, 1.0 / D)
    t = wp.tile([128, D], F32, tag=f"lnc{uid}")
    nc.vector.tensor_scalar_sub(t, pin, mu)
    sq = wp.tile([128, D], F32, tag=f"lnsq{uid}")
    vs = sp.tile([128, 1], F32, tag=f"vs{uid}")
    nc.scalar.activation(sq, t, AF.Square, bias=zb, accum_out=vs)
    lv = sp.tile([128, 1], F32, tag=f"lv{uid}")
    nc.scalar.activation(lv, vs, AF.Ln, scale=1.0 / D, bias=epsb)
    rstd = sp.tile([128, 1], F32, tag=f"rstd{uid}")
    nc.scalar.activation(rstd, lv, AF.Exp, scale=-0.5, bias=zb)
    nc.vector.scalar_tensor_tensor(out_ap, t, rstd, g_ap, op0=OP.mult, op1=OP.mult)
    nc.vector.tensor_add(out_ap, out_ap, b_ap)


def _patch_act_tables():
    """Steer the ACT-table chooser to the one set containing every function
    we use (exp/ln/abs/square/relu/prelu/identity), avoiding per-LN table
    reloads.  Indices (act_func_set_id) are preserved; other sets just lose
    these functions so the fixpoint can't pick them."""
    import concourse.bacc as bacc_mod
    import concourse.hw_specs as hw_specs
    if getattr(bacc_mod, "_act_tables_patched", False):
        return
    orig = hw_specs.get_activation_tables
    mine = {AF.Exp, AF.Ln, AF.Abs, AF.Square, AF.Relu, AF.Identity, AF.Prelu}

    def patched(arch):
        t = orig(arch)
        return {name: (s if name == "natural_log_exp_and_others" else (s - mine))
                for name, s in t.items()}

    bacc_mod.get_activation_tables = patched
    bacc_mod._act_tables_patched = True


def build_nc():
    _patch_act_tables()
    nc = bacc.Bacc("TRN2", target_bir_lowering=False, debug=False)

    xT = nc.dram_tensor("xT", [2, BL * N], F32, kind="ExternalInput")
    wl = nc.dram_tensor("wl", [HID, L * HID], F16, kind="ExternalInput")
    wr = nc.dram_tensor("wr", [HID, L * HID], F16, kind="ExternalInput")
    pw = nc.dram_tensor("pw", [HID, L * HID], F16, kind="ExternalInput")
    et = nc.dram_tensor("et", [HID, L * K * K], F16, kind="ExternalInput")
    etd = nc.dram_tensor("etd", [HID, L * K], F16, kind="ExternalInput")
    ab16 = nc.dram_tensor("ab16", [HID, L * H], F16, kind="ExternalInput")
    brow = nc.dram_tensor("brow", [1, 10 * HID], F16, kind="ExternalInput")
    mw1 = nc.dram_tensor("mw1", [2, HID], F32, kind="ExternalInput")
    mw2 = nc.dram_tensor("mw2", [HID, HID], F16, kind="ExternalInput")
    lnr = nc.dram_tensor("lnr", [HID, 12 * HID], F16, kind="ExternalInput")
    ow = nc.dram_tensor("ow", [HID, 1], F32, kind="ExternalInput")
    ob = nc.dram_tensor("ob", [1, 1], F32, kind="ExternalInput")
    out = nc.dram_tensor("out", [BL, 1], F32, kind="ExternalOutput")

    with tile.TileContext(nc) as tc, ExitStack() as ctx:
        cp = ctx.enter_context(tc.tile_pool(name="const", bufs=1))
        pp = ctx.enter_context(tc.tile_pool(name="perb", bufs=1))
        wp = ctx.enter_context(tc.tile_pool(name="work", bufs=3))
        sp = ctx.enter_context(tc.tile_pool(name="small", bufs=4))
        mb = ctx.enter_context(tc.tile_pool(name="mb", bufs=6))
        ps = ctx.enter_context(tc.tile_pool(name="ps", bufs=2, space="PSUM"))
        pt = ctx.enter_context(tc.tile_pool(name="pt", bufs=1, space="PSUM"))
        pg = ctx.enter_context(tc.tile_pool(name="pg", bufs=2, space="PSUM"))
        pd = ctx.enter_context(tc.tile_pool(name="pd", bufs=1, space="PSUM"))
        pm = ctx.enter_context(tc.tile_pool(name="pm", bufs=1, space="PSUM"))

        # ---- load constants ----
        def load(dram, shape, name, dt=F32):
            t = cp.tile(shape, dt, tag=name)
            nc.sync.dma_start(t[:], dram[:])
            return t

        xT_s = load(xT, [2, BL * N], "xT", F32)
        mw1_s = load(mw1, [2, HID], "mw1", F32)
        brow_s = load(brow, [1, 10 * HID], "brow", F16)
        lnr_s = load(lnr, [HID, 12 * HID], "lnr", F16)
        mw2_s = load(mw2, [HID, HID], "mw2", F16)
        wl_s = load(wl, [HID, L * HID], "wl", F16)
        wr_s = load(wr, [HID, L * HID], "wr", F16)
        pw_s = load(pw, [HID, L * HID], "pw", F16)
        et_s = load(et, [HID, L * K * K], "et", F16)
        etd_s = load(etd, [HID, L * K], "etd", F16)
        ab16_s = load(ab16, [HID, L * H], "ab16", F16)
        ow_s = load(ow, [HID, 1], "ow", F32)
        ob_s = load(ob, [1, 1], "ob", F32)

        ident = cp.tile([128, 128], F32, tag="ident")
        make_identity(nc, ident[:])
        ident16 = cp.tile([128, 128], F16, tag="ident16")
        nc.vector.tensor_copy(ident16, ident)
        ones16_r = cp.tile([1, N], F16, tag="ones16_r")
        nc.gpsimd.memset(ones16_r[:], 1.0)
        zb = cp.tile([128, 1], F32, tag="zb")
        nc.gpsimd.memset(zb[:], 0.0)
        epsb = cp.tile([128, 1], F32, tag="epsb")
        nc.gpsimd.memset(epsb[:], EPS)

        # per-graph persistent tiles
        xla = pp.tile([128, BL * H * AUG], F16, tag="xla")  # [xl | 1] per head
        nc.gpsimd.memset(xla[:], 1.0)
        hT_s = pp.tile([HID, BL * N], F16, tag="hT")
        hT2_s = pp.tile([HID, BL * N], F16, tag="hT2")
        xr_s = pp.tile([HID, BL * N], F16, tag="xr")
        u_s = pp.tile([HID, BL * K * N], F16, tag="u")
        es_s = pp.tile([128, BL * N * H], F16, tag="es")  # exp scores [i,(j,h)]

        def lnv(i):  # replicated LN vector slice [128, 128]
            return lnr_s[:, i * HID:(i + 1) * HID]


        # ======== input MLP ========
        for b in range(BL):
            p1 = pg.tile([128, HID], F32, tag="pg")
            nc.tensor.matmul(p1, xT_s[:, b * N:(b + 1) * N], mw1_s[:], start=True, stop=False)
            nc.tensor.matmul(p1, ones16_r[:], brow_s[:, 0:HID], start=False, stop=True)
            h1 = wp.tile([128, HID], F32, tag="h1")
            _ln_free(nc, wp, sp, p1[:], lnv(LN1G), lnv(LN1B), h1[:], "a", zb, epsb)
            h1r = wp.tile([128, HID], F32, tag="h1r")
            nc.scalar.activation(h1r, h1, AF.Relu, bias=zb)
            ptr = pt.tile([128, 128], F32, tag="ptr")
            nc.tensor.transpose(ptr, h1r[:], ident[:])
            h1T = wp.tile([128, HID], F16, tag="h1T")
            nc.scalar.activation(h1T, ptr, AF.Identity, bias=zb)
            p2 = pg.tile([128, HID], F32, tag="pg")
            nc.tensor.matmul(p2, h1T[:], mw2_s[:], start=True, stop=False)
            nc.tensor.matmul(p2, ones16_r[:], brow_s[:, HID:2 * HID], start=False, stop=True)
            hb = wp.tile([128, HID], F32, tag="hmlp")
            _ln_free(nc, wp, sp, p2[:], lnv(LN2G), lnv(LN2B), hb[:], "b", zb, epsb)
            ptr2 = pt.tile([128, 128], F32, tag="ptr")
            nc.tensor.transpose(ptr2, hb[:], ident[:])
            nc.scalar.activation(hT_s[:, b * N:(b + 1) * N], ptr2,
                                 AF.Identity, bias=zb)

        # ======== GATv2 layers ========
        # The engines execute their queues in program order, so the two
        # graphs' work is emitted interleaved stage-by-stage: while graph 0's
        # lrelu runs on ScalarE, graph 1's message build runs on DVE, and
        # score matmuls from both fill the tensor engine.
        for l in range(L):
            wls = wl_s[:, l * HID:(l + 1) * HID]
            wrs = wr_s[:, l * HID:(l + 1) * HID]
            pws = pw_s[:, l * HID:(l + 1) * HID]
            ab16s = ab16_s[:, l * H:(l + 1) * H]
            ets = et_s[:, l * K * K:(l + 1) * K * K]
            etds = etd_s[:, l * K:(l + 1) * K]
            hTcur, hTnxt = (hT_s, hT2_s) if l % 2 == 0 else (hT2_s, hT_s)
            V = []
            for b in range(BL):
                V.append(dict(
                    hTb=hTcur[:, b * N:(b + 1) * N],
                    hTn=hTnxt[:, b * N:(b + 1) * N],
                    xrb=xr_s[:, b * N:(b + 1) * N],
                    ub=u_s[:, b * K * N:(b + 1) * K * N],
                    esb=es_s[:, b * N * H:(b + 1) * N * H],
                    xlab=xla[:, b * H * AUG:(b + 1) * H * AUG]))

            # xl (natural layout, with bias) -> augmented o-matmul rhs
            for b in range(BL):
                v = V[b]
                pxl = pg.tile([128, HID], F32, tag="pg")
                nc.tensor.matmul(pxl, v["hTb"], wls, start=True, stop=False)
                nc.tensor.matmul(pxl, ones16_r[:], brow_s[:, (2 + l) * HID:(3 + l) * HID], start=False, stop=True)
                nc.scalar.activation(
                    v["xlab"].rearrange("i (h q) -> i h q", q=AUG)[:, :, 0:C],
                    pxl.rearrange("i (h c) -> i h c", c=C),
                    AF.Identity, bias=zb)

            # xr [hc, j] = Wr^T h_T + bl  (bias attached to xr side)
            for b in range(BL):
                v = V[b]
                pxr = pg.tile([128, HID], F32, tag="pg")
                nc.tensor.matmul(pxr, wrs, v["hTb"], start=True, stop=False)
                nc.tensor.matmul(pxr, brow_s[:, (2 + l) * HID:(3 + l) * HID],
                                 ones16_r[:], start=False, stop=True)
                nc.scalar.activation(v["xrb"], pxr, AF.Identity, bias=zb)

            # u[hc, (q, i)] = (Wl^T h_T)[hc, i] + et[hc, (pos(i), q)]
            for b in range(BL):
                v = V[b]
                pxt = pg.tile([128, HID], F32, tag="pg")
                nc.tensor.matmul(pxt, wls, v["hTb"], start=True, stop=True)
                nc.vector.scalar_tensor_tensor(
                    v["ub"].rearrange("k (q p t) -> k q p t", q=K, p=K),
                    pxt.rearrange("k (o p t) -> k o p t", o=1, p=K)
                        .broadcast_to((HID, K, K, NO)),
                    0.0,
                    ets.rearrange("k (q p o) -> k q p o", q=K, o=1)
                        .broadcast_to((HID, K, K, NO)),
                    op0=OP.add, op1=OP.add)

            # diagonal: dmw = u-diag + xr, dmd = dmw + etdelta; scores + exp
            delts = [None] * BL
            for b in range(BL):
                v = V[b]
                dmwd = wp.tile([128, 2 * N], F16, tag="dmwd")
                for q in range(K):
                    nc.vector.scalar_tensor_tensor(
                        dmwd[:, q * NO:(q + 1) * NO],
                        v["ub"][:, q * (N + NO):q * (N + NO) + NO],
                        0.0, v["xrb"][:, q * NO:(q + 1) * NO],
                        op0=OP.add, op1=OP.add)
                nc.vector.tensor_tensor(
                    dmwd[:, N:2 * N].rearrange("k (q t) -> k q t", q=K),
                    dmwd[:, 0:N].rearrange("k (q t) -> k q t", q=K),
                    etds.rearrange("k (q o) -> k q o", o=1)
                        .broadcast_to((HID, K, NO)),
                    op=OP.add)
                ldm = wp.tile([128, 2 * N], F16, tag="ldm")
                nc.scalar.activation(ldm, dmwd, AF.Prelu, bias=zb, alpha=NEG)
                psd = pd.tile([128, 2 * H], F32, tag="psd")
                nc.tensor.matmul(psd[:, 0:H], ldm[:, 0:N], ab16s,
                                 start=True, stop=True, skip_group_check=True)
                nc.tensor.matmul(psd[:, H:2 * H], ldm[:, N:2 * N], ab16s,
                                 start=True, stop=True, skip_group_check=True)
                esd = sp.tile([128, 2 * H], F16, tag="esd")
                nc.scalar.activation(esd, psd, AF.Exp, bias=zb)
                delt = sp.tile([128, H], F16, tag=f"delt{b}")
                nc.vector.tensor_tensor(delt, esd[:, H:2 * H], esd[:, 0:H],
                                        op=OP.subtract)
                delts[b] = delt

            # ---- message blocks over target nodes j, graphs interleaved ----
            psbs = [None] * BL
            for half in range(2):
                for b in range(BL):
                    psbs[b] = ps.tile([128, (N // 2) * H], F32, tag="psb", name=f"psb{b}")
                for blk in range(NBB // 2):
                    j0 = half * (N // 2) + blk * JBB
                    q = j0 // NO
                    path = {2: 2, 5: 2, 7: 4}.get(half * (NBB // 2) + blk, 1)
                    for b in range(BL):
                        v = V[b]
                        psb = psbs[b]
                        if path == 2:
                            # PE builds m into PSUM, lrelu on ScalarE
                            for sub in range(2):
                                j0s = j0 + sub * (JBB // 2)
                                pmb = pm.tile([128, (JBB // 2) * N], F32,
                                              tag="pmb")
                                pm3 = pmb.rearrange("k (j i) -> k j i",
                                                    j=JBB // 2)
                                for g in range(2):
                                    g0 = g * (JBB // 4)
                                    nc.tensor.matmul(
                                        pm3[:, g0:g0 + JBB // 4, :],
                                        ident16[:],
                                        v["ub"][:, q * N:(q + 1) * N]
                                            .rearrange("k (o i) -> k o i", o=1)
                                            .broadcast_to((HID, JBB // 4, N)),
                                        start=True, stop=False,
                                        skip_group_check=True)
                                    nc.tensor.matmul(
                                        pm3[:, g0:g0 + JBB // 4, :],
                                        ident16[:],
                                        v["xrb"][:, j0s + g0:j0s + g0 + JBB // 4]
                                            .rearrange("k (j o) -> k j o", o=1)
                                            .broadcast_to((HID, JBB // 4, N)),
                                        start=False, stop=True,
                                        skip_group_check=True)
                                ma8 = mb.tile([128, (JBB // 2) * N], F16,
                                              tag="ma8")
                                nc.scalar.activation(ma8, pmb, AF.Prelu,
                                                     bias=zb, alpha=NEG)
                                for t in range(JBB // 2):
                                    jl = blk * JBB + sub * (JBB // 2) + t
                                    nc.tensor.matmul(
                                        psb[:, jl * H:(jl + 1) * H],
                                        ma8[:, t * N:(t + 1) * N], ab16s,
                                        start=True, stop=True,
                                        skip_group_check=True)
                        else:
                            mp16 = mb.tile([128, JBB * N], F16, tag="mp")
                            nc.vector.tensor_tensor(
                                mp16.rearrange("k (j i) -> k j i", j=JBB),
                                v["ub"][:, q * N:(q + 1) * N]
                                    .rearrange("k (o i) -> k o i", o=1)
                                    .broadcast_to((HID, JBB, N)),
                                v["xrb"][:, j0:j0 + JBB]
                                    .rearrange("k (j o) -> k j o", o=1)
                                    .broadcast_to((HID, JBB, N)),
                                op=OP.add)
                            ma16 = mb.tile([128, JBB * N], F16, tag="ma")
                            if path == 4:
                                nc.vector.scalar_tensor_tensor(
                                    ma16[:], mp16[:], float(NEG), mp16[:],
                                    op0=OP.mult, op1=OP.max)
                            else:
                                nc.scalar.activation(ma16, mp16, AF.Prelu,
                                                     bias=zb, alpha=NEG)
                            for t in range(JBB):
                                jl = blk * JBB + t
                                nc.tensor.matmul(
                                    psb[:, jl * H:(jl + 1) * H],
                                    ma16[:, t * N:(t + 1) * N], ab16s,
                                    start=True, stop=True,
                                    skip_group_check=True)
                        if blk == NBB // 2 - 1:
                            nc.scalar.activation(
                                v["esb"][:, half * (N // 2) * H:
                                         (half + 1) * (N // 2) * H],
                                psb, AF.Exp, bias=zb)

            # aggregate + normalizer + diagonal correction
            osbs = [None] * BL
            for b in range(BL):
                v = V[b]
                po = pg.tile([128, H * AUG], F32, tag="pg")
                es3 = v["esb"].rearrange("i (j h) -> i j h", h=H)
                for h in range(H):
                    nc.tensor.matmul(
                        po[:, h * AUG:(h + 1) * AUG],
                        es3[:, :, h],
                        v["xlab"][:, h * AUG:(h + 1) * AUG],
                        start=True, stop=True)
                po3 = po.rearrange("j (h q) -> j h q", q=AUG)
                delt = delts[b]
                dtmp = wp.tile([128, H * C], F16, tag="dtmp")
                nc.vector.tensor_tensor(
                    dtmp.rearrange("j (h c) -> j h c", c=C),
                    delt.rearrange("j (h o) -> j h o", o=1)
                        .broadcast_to((128, H, C)),
                    v["xlab"].rearrange("i (h q) -> i h q", q=AUG)[:, :, 0:C],
                    op=OP.mult)
                nc.vector.tensor_add(
                    po3[:, :, 0:C], po3[:, :, 0:C],
                    dtmp.rearrange("j (h c) -> j h c", c=C))
                nc.vector.tensor_add(
                    po3[:, :, 16:17], po3[:, :, 16:17],
                    delt.rearrange("j (h o) -> j h o", o=1))
                zc = sp.tile([128, H], F32, tag="zc")
                nc.vector.tensor_copy(
                    zc.rearrange("j (h o) -> j h o", o=1),
                    po3[:, :, 16:17])
                rz = sp.tile([128, H], F32, tag="rz")
                nc.vector.reciprocal(rz, zc)
                o_sb = wp.tile([128, HID], F32, tag=f"osb{b}")
                nc.vector.tensor_mul(
                    o_sb.rearrange("j (h c) -> j h c", c=C),
                    po3[:, :, 0:C],
                    rz.rearrange("j (h o) -> j h o", o=1).broadcast_to((128, H, C)))
                osbs[b] = o_sb

            # projection + LN + relu + residual
            for b in range(BL):
                v = V[b]
                pto = pt.tile([128, 128], F32, tag="ptr")
                nc.tensor.transpose(pto, osbs[b][:], ident[:])
                oT = wp.tile([128, HID], F16, tag="oT")
                nc.scalar.activation(oT, pto, AF.Identity, bias=zb)
                ppj = pg.tile([128, HID], F32, tag="pg")
                nc.tensor.matmul(ppj, oT[:], pws, start=True, stop=False)
                nc.tensor.matmul(ppj, ones16_r[:], brow_s[:, (6 + l) * HID:(7 + l) * HID], start=False, stop=True)
                lno = wp.tile([128, HID], F32, tag=f"lno{b}")
                _ln_free(nc, wp, sp, ppj[:], lnv(LNG0 + l), lnv(LNB0 + l), lno[:], f"c{b}", zb, epsb)
                rl = wp.tile([128, HID], F32, tag=f"rl{b}")
                nc.scalar.activation(rl, lno, AF.Relu, bias=zb)
                ptr2 = pt.tile([128, 128], F32, tag="ptr")
                nc.tensor.transpose(ptr2, rl[:], ident[:])
                rlT = wp.tile([128, HID], F16, tag=f"rlT{b}")
                nc.scalar.activation(rlT, ptr2, AF.Identity, bias=zb)
                nc.vector.tensor_tensor(v["hTn"], rlT[:], v["hTb"], op=OP.add)

        # ======== pooling + head ========
        hTfin = hT_s if L % 2 == 0 else hT2_s
        for b in range(BL):
            hagg = sp.tile([128, 1], F32, tag="hagg")
            nc.vector.tensor_reduce(hagg, hTfin[:, b * N:(b + 1) * N],
                                    axis=AX.X, op=OP.add)
            pr = pg.tile([1, 1], F32, tag="pg")
            nc.tensor.matmul(pr, hagg[:], ow_s[:], start=True, stop=True)
            res = sp.tile([1, 1], F32, tag="res")
            nc.scalar.activation(res, pr, AF.Identity, bias=ob_s[0:1, 0:1])
            nc.sync.dma_start(out[b:b + 1, :], res[:])

    nc.compile()
    return nc


def pack_inputs(inputs):
    """Full model inputs -> per-core in_maps (host-side shard + re-layout)."""
    f = {k: np.asarray(v, dtype=np.float32) if k != "cat" else np.asarray(v)
         for k, v in inputs.items()}

    # the kernel exploits the orbit structure of cat; verify it holds
    cat = np.asarray(f["cat"], dtype=np.int64)
    pos_ = np.arange(N) // NO
    i_, j_ = np.arange(N)[:, None], np.arange(N)[None, :]
    cat_exp = np.where(i_ == j_, K * K + pos_[:, None],
                       pos_[:, None] * K + pos_[None, :])
    assert np.array_equal(cat, cat_exp), "cat does not match orbit structure"

    att = f["att"]
    abk = np.zeros((HID, L * H), np.float32)
    for l in range(L):
        for h in range(H):
            abk[h * C:(h + 1) * C, l * H + h] = att[l, h]

    pb_eff = np.stack([f["cb"][l] @ f["pW"][l] + f["pb"][l] for l in range(L)])

    # edge-category transforms: e20[l] = emb @ We[l]  -> [20, HID]
    # off-diag cat(p source, q target) = p*K+q; diag cat = K*K+q
    et = np.zeros((HID, L * K * K), np.float16)
    etd = np.zeros((HID, L * K), np.float16)
    for l in range(L):
        e20 = f["emb"] @ f["We"][l]                 # [20, HID]
        for q in range(K):
            for p in range(K):
                et[:, l * K * K + q * K + p] = e20[p * K + q]
            etd[:, l * K + q] = e20[K * K + q] - e20[q * K + q]

    lnvecs = [f["ln1_g"], f["ln1_b"], f["ln2_g"], f["ln2_b"],
              *[f["lng"][l] for l in range(L)], *[f["lnb"][l] for l in range(L)]]
    lnr = np.ascontiguousarray(
        np.broadcast_to(np.concatenate(lnvecs)[None, :], (HID, 12 * HID)))

    def stackw(w):  # [L, k, hc] -> [k, L*hc] so sbuf slice l is W[l][k, hc]
        return np.ascontiguousarray(
            w.transpose(1, 0, 2).reshape(HID, L * HID)).astype(np.float16)

    shared = {
        "wl": stackw(f["Wl"]), "wr": stackw(f["Wr"]), "pw": stackw(f["pW"]),
        "et": et, "etd": etd, "ab16": abk.astype(np.float16),
        "brow": np.concatenate([f["mlp_b1"], f["mlp_b2"],
                                f["bl"].ravel(), pb_eff.ravel()])
            .reshape(1, 10 * HID).astype(np.float16),
        "mw1": f["mlp_w1"], "mw2": f["mlp_w2"].astype(np.float16),
        "lnr": lnr.astype(np.float16), "ow": f["out_w"].reshape(HID, 1),
        "ob": f["out_b"].reshape(1, 1),
    }
    in_maps = []
    for c in range(NCORES):
        xTc = np.ascontiguousarray(
            f["x"][c * BL:(c + 1) * BL].transpose(2, 0, 1)).reshape(2, BL * N)
        m = dict(shared)
        m["xT"] = xTc
        in_maps.append(m)
    return in_maps


_NC = None
LAST_EXEC_NS = None


def kernel(**inputs) -> np.ndarray:
    global _NC, LAST_EXEC_NS
    from concourse.bass_utils import run_bass_kernel_spmd
    if _NC is None:
        _NC = build_nc()
    import os
    in_maps = pack_inputs(inputs)
    trace = bool(os.environ.get("KERNEL_TRACE"))
    r = run_bass_kernel_spmd(_NC, in_maps, core_ids=list(range(NCORES)),
                             trace=trace)
    LAST_EXEC_NS = r.exec_time_ns
    out = np.concatenate([r.results[c]["out"] for c in range(NCORES)], axis=0)
    return out.astype(np.float32)


# revision 28
# speedup vs baseline: 1.0621x; 1.0621x over previous
"""GATv2 model kernel for Trainium2 (Bass/Tile), data-parallel over batch on 8 cores.

Model (per graph b): input MLP -> 4 GATv2 layers (dense N^2 attention with
edge features) -> sum-pool -> linear head.  B=16, N=128, HID=128, H=8, C=16.

Key structural fact: cat[i,j] takes only K*K+K = 20 distinct values (K=4
orbits of 32 consecutive nodes; diagonal i==j uses its own 4 categories).
So e_feat has 20 distinct rows and the whole e-transform collapses to a
host-side [20, HID] table.  Per-core layout (2 graphs per core):

  - h kept fp32 [node, hid]; hT/xlT/xrT produced per layer via PE transpose +
    fp16 copies; all weight matmuls run fp16 (1 cyc/row vs 4 for fp32).
  - u[hc, (q, i)] = xlT + e_offdiag(pos(i), q) built by one DVE op; message
    m for a 16-target block = one DVE STT (u + xr broadcast, fp16), then
    ScalarE Prelu(alpha=0.2) gives lrelu(m) -- scores are att.lrelu(m)
    computed exactly by one per-target fp16 matmul into its own 8-column
    PSUM range (no accumulation chain), then one exp -> fp16 scores.
  - diagonal (i==j category) fixed exactly post-hoc: correct/wrong diagonal
    scores computed per target in [j, h] layout (2 small matmuls + exp),
    delta = exp(sd)-exp(sw) patches the aggregation output po and the
    softmax normalizer column -- pointwise in j, no scatter.
  - aggregation via per-head matmuls with [xl | 1] (fp16) producing the
    unnormalized output and softmax normalizer Z in one PSUM tile.
  - LayerNorm rsqrt via exp(-0.5*ln(var+eps)); an ACT-table patch keeps all
    activations in the one table containing exp+ln (no per-LN table reloads).
"""

import numpy as np
from contextlib import ExitStack

import concourse.bacc as bacc
import concourse.bass as bass
import concourse.tile as tile
from concourse import mybir
from concourse.masks import make_identity

F32 = mybir.dt.float32
F16 = mybir.dt.float16
AF = mybir.ActivationFunctionType
OP = mybir.AluOpType
AX = mybir.AxisListType

B, N, HID, H, C, L, K = 16, 128, 128, 8, 16, 4, 4
NCORES = 8
BL = B // NCORES          # graphs per core
NEG = 0.2                 # leaky relu slope
EPS = 1e-5
AUG = 17                  # head dim + 1 (softmax normalizer column)
NO = N // K               # nodes per orbit (32)
JBB = 16                  # targets per message block
NBB = N // JBB

# LN vector slots in the replicated-params tile
LN1G, LN1B, LN2G, LN2B = 0, 1, 2, 3
LNG0, LNB0 = 4, 8


def _ln_free(nc, wp, sp, pin, g_ap, b_ap, out_ap, uid, zb, epsb):
    """LayerNorm along the free dim of pin [128, D] -> out_ap (SBUF)."""
    D = pin.shape[-1]
    mu = sp.tile([128, 1], F32, tag=f"mu{uid}")
    nc.vector.tensor_reduce(mu, pin, axis=AX.X, op=OP.add)
    nc.vector.tensor_scalar_mul(mu, mu, 1.0 / D)
    t = wp.tile([128, D], F32, tag=f"lnc{uid}")
    nc.vector.tensor_scalar_sub(t, pin, mu)
    sq = wp.tile([128, D], F32, tag=f"lnsq{uid}")
    vs = sp.tile([128, 1], F32, tag=f"vs{uid}")
    nc.scalar.activation(sq, t, AF.Square, bias=zb, accum_out=vs)
    lv = sp.tile([128, 1], F32, tag=f"lv{uid}")
    nc.scalar.activation(lv, vs, AF.Ln, scale=1.0 / D, bias=epsb)
    rstd = sp.tile([128, 1], F32, tag=f"rstd{uid}")
    nc.scalar.activation(rstd, lv, AF.Exp, scale=-0.5, bias=zb)
    nc.vector.scalar_tensor_tensor(out_ap, t, rstd, g_ap, op0=OP.mult, op1=OP.mult)
    nc.vector.tensor_add(out_ap, out_ap, b_ap)


def _patch_act_tables():
    """Steer the ACT-table chooser to the one set containing every function
    we use (exp/ln/abs/square/relu/prelu/identity), avoiding per-LN table
    reloads.  Indices (act_func_set_id) are preserved; other sets just lose
    these functions so the fixpoint can't pick them."""
    import concourse.bacc as bacc_mod
    import concourse.hw_specs as hw_specs
    if getattr(bacc_mod, "_act_tables_patched", False):
        return
    orig = hw_specs.get_activation_tables
    mine = {AF.Exp, AF.Ln, AF.Abs, AF.Square, AF.Relu, AF.Identity, AF.Prelu}

    def patched(arch):
        t = orig(arch)
        return {name: (s if name == "natural_log_exp_and_others" else (s - mine))
                for name, s in t.items()}

    bacc_mod.get_activation_tables = patched
    bacc_mod._act_tables_patched = True


def build_nc():
    _patch_act_tables()
    nc = bacc.Bacc("TRN2", target_bir_lowering=False, debug=False)

    xT = nc.dram_tensor("xT", [2, BL * N], F32, kind="ExternalInput")
    wl = nc.dram_tensor("wl", [HID, L * HID], F16, kind="ExternalInput")
    wr = nc.dram_tensor("wr", [HID, L * HID], F16, kind="ExternalInput")
    pw = nc.dram_tensor("pw", [HID, L * HID], F16, kind="ExternalInput")
    et = nc.dram_tensor("et", [HID, L * K * K], F16, kind="ExternalInput")
    etd = nc.dram_tensor("etd", [HID, L * K], F16, kind="ExternalInput")
    ab16 = nc.dram_tensor("ab16", [HID, L * H], F16, kind="ExternalInput")
    brow = nc.dram_tensor("brow", [1, 10 * HID], F16, kind="ExternalInput")
    mw1 = nc.dram_tensor("mw1", [2, HID], F32, kind="ExternalInput")
    mw2 = nc.dram_tensor("mw2", [HID, HID], F16, kind="ExternalInput")
    lnr = nc.dram_tensor("lnr", [HID, 12 * HID], F16, kind="ExternalInput")
    ow = nc.dram_tensor("ow", [HID, 1], F32, kind="ExternalInput")
    ob = nc.dram_tensor("ob", [1, 1], F32, kind="ExternalInput")
    out = nc.dram_tensor("out", [BL, 1], F32, kind="ExternalOutput")

    with tile.TileContext(nc) as tc, ExitStack() as ctx:
        cp = ctx.enter_context(tc.tile_pool(name="const", bufs=1))
        pp = ctx.enter_context(tc.tile_pool(name="perb", bufs=1))
        hp = ctx.enter_context(tc.tile_pool(name="hpool", bufs=2))
        wp = ctx.enter_context(tc.tile_pool(name="work", bufs=3))
        sp = ctx.enter_context(tc.tile_pool(name="small", bufs=4))
        mb = ctx.enter_context(tc.tile_pool(name="mb", bufs=6))
        ps = ctx.enter_context(tc.tile_pool(name="ps", bufs=2, space="PSUM"))
        pt = ctx.enter_context(tc.tile_pool(name="pt", bufs=1, space="PSUM"))
        pg = ctx.enter_context(tc.tile_pool(name="pg", bufs=2, space="PSUM"))
        pd = ctx.enter_context(tc.tile_pool(name="pd", bufs=1, space="PSUM"))
        pm = ctx.enter_context(tc.tile_pool(name="pm", bufs=1, space="PSUM"))

        # ---- load constants ----
        def load(dram, shape, name, dt=F32):
            t = cp.tile(shape, dt, tag=name)
            nc.sync.dma_start(t[:], dram[:])
            return t

        xT_s = load(xT, [2, BL * N], "xT", F32)
        mw1_s = load(mw1, [2, HID], "mw1", F32)
        brow_s = load(brow, [1, 10 * HID], "brow", F16)
        lnr_s = load(lnr, [HID, 12 * HID], "lnr", F16)
        mw2_s = load(mw2, [HID, HID], "mw2", F16)
        wl_s = load(wl, [HID, L * HID], "wl", F16)
        wr_s = load(wr, [HID, L * HID], "wr", F16)
        pw_s = load(pw, [HID, L * HID], "pw", F16)
        et_s = load(et, [HID, L * K * K], "et", F16)
        etd_s = load(etd, [HID, L * K], "etd", F16)
        ab16_s = load(ab16, [HID, L * H], "ab16", F16)
        ow_s = load(ow, [HID, 1], "ow", F32)
        ob_s = load(ob, [1, 1], "ob", F32)

        ident = cp.tile([128, 128], F32, tag="ident")
        make_identity(nc, ident[:])
        ident16 = cp.tile([128, 128], F16, tag="ident16")
        nc.vector.tensor_copy(ident16, ident)
        ones16_r = cp.tile([1, N], F16, tag="ones16_r")
        nc.gpsimd.memset(ones16_r[:], 1.0)
        ones_c = cp.tile([128, 1], F32, tag="ones_c")
        nc.gpsimd.memset(ones_c[:], 1.0)
        zb = cp.tile([128, 1], F32, tag="zb")
        nc.gpsimd.memset(zb[:], 0.0)
        epsb = cp.tile([128, 1], F32, tag="epsb")
        nc.gpsimd.memset(epsb[:], EPS)

        # per-graph persistent tiles
        xla = pp.tile([128, BL * H * AUG], F16, tag="xla")  # [xl | 1] per head
        nc.gpsimd.memset(xla[:], 1.0)
        hT_s = pp.tile([HID, BL * N], F16, tag="hT")
        xr_s = pp.tile([HID, BL * N], F16, tag="xr")
        u_s = pp.tile([HID, BL * K * N], F16, tag="u")
        es_s = pp.tile([128, BL * N * H], F16, tag="es")  # exp scores [i,(j,h)]

        h_cur = [None] * BL

        def lnv(i):  # replicated LN vector slice [128, 128]
            return lnr_s[:, i * HID:(i + 1) * HID]


        # ======== input MLP ========
        for b in range(BL):
            p1 = pg.tile([128, HID], F32, tag="pg")
            nc.tensor.matmul(p1, xT_s[:, b * N:(b + 1) * N], mw1_s[:], start=True, stop=False)
            nc.tensor.matmul(p1, ones16_r[:], brow_s[:, 0:HID], start=False, stop=True)
            h1 = wp.tile([128, HID], F32, tag="h1")
            _ln_free(nc, wp, sp, p1[:], lnv(LN1G), lnv(LN1B), h1[:], "a", zb, epsb)
            h1r = wp.tile([128, HID], F32, tag="h1r")
            nc.scalar.activation(h1r, h1, AF.Relu, bias=zb)
            ptr = pt.tile([128, 128], F32, tag="ptr")
            nc.tensor.transpose(ptr, h1r[:], ident[:])
            h1T = wp.tile([128, HID], F16, tag="h1T")
            nc.scalar.activation(h1T, ptr, AF.Identity, bias=zb)
            p2 = pg.tile([128, HID], F32, tag="pg")
            nc.tensor.matmul(p2, h1T[:], mw2_s[:], start=True, stop=False)
            nc.tensor.matmul(p2, ones16_r[:], brow_s[:, HID:2 * HID], start=False, stop=True)
            hb = hp.tile([128, HID], F32, tag=f"h{b}")
            _ln_free(nc, wp, sp, p2[:], lnv(LN2G), lnv(LN2B), hb[:], "b", zb, epsb)
            h_cur[b] = hb

        # ======== GATv2 layers ========
        # The engines execute their queues in program order, so the two
        # graphs' work is emitted interleaved stage-by-stage: while graph 0's
        # lrelu runs on ScalarE, graph 1's message build runs on DVE, and
        # score matmuls from both fill the tensor engine.
        for l in range(L):
            wls = wl_s[:, l * HID:(l + 1) * HID]
            wrs = wr_s[:, l * HID:(l + 1) * HID]
            pws = pw_s[:, l * HID:(l + 1) * HID]
            ab16s = ab16_s[:, l * H:(l + 1) * H]
            ets = et_s[:, l * K * K:(l + 1) * K * K]
            etds = etd_s[:, l * K:(l + 1) * K]
            V = []
            for b in range(BL):
                V.append(dict(
                    hb=h_cur[b],
                    hTb=hT_s[:, b * N:(b + 1) * N],
                    xrb=xr_s[:, b * N:(b + 1) * N],
                    ub=u_s[:, b * K * N:(b + 1) * K * N],
                    esb=es_s[:, b * N * H:(b + 1) * N * H],
                    xlab=xla[:, b * H * AUG:(b + 1) * H * AUG]))

            # h_T (fp16)
            for b in range(BL):
                v = V[b]
                ptr = pt.tile([128, 128], F32, tag="ptr")
                nc.tensor.transpose(ptr, v["hb"][:], ident[:])
                nc.scalar.activation(v["hTb"], ptr, AF.Identity, bias=zb)

            # xl (natural layout, with bias) -> augmented o-matmul rhs
            for b in range(BL):
                v = V[b]
                pxl = pg.tile([128, HID], F32, tag="pg")
                nc.tensor.matmul(pxl, v["hTb"], wls, start=True, stop=False)
                nc.tensor.matmul(pxl, ones16_r[:], brow_s[:, (2 + l) * HID:(3 + l) * HID], start=False, stop=True)
                nc.scalar.activation(
                    v["xlab"].rearrange("i (h q) -> i h q", q=AUG)[:, :, 0:C],
                    pxl.rearrange("i (h c) -> i h c", c=C),
                    AF.Identity, bias=zb)

            # xr [hc, j] = Wr^T h_T + bl  (bias attached to xr side)
            for b in range(BL):
                v = V[b]
                pxr = pg.tile([128, HID], F32, tag="pg")
                nc.tensor.matmul(pxr, wrs, v["hTb"], start=True, stop=False)
                nc.tensor.matmul(pxr, brow_s[:, (2 + l) * HID:(3 + l) * HID],
                                 ones16_r[:], start=False, stop=True)
                nc.scalar.activation(v["xrb"], pxr, AF.Identity, bias=zb)

            # u[hc, (q, i)] = (Wl^T h_T)[hc, i] + et[hc, (pos(i), q)]
            for b in range(BL):
                v = V[b]
                pxt = pg.tile([128, HID], F32, tag="pg")
                nc.tensor.matmul(pxt, wls, v["hTb"], start=True, stop=True)
                nc.vector.scalar_tensor_tensor(
                    v["ub"].rearrange("k (q p t) -> k q p t", q=K, p=K),
                    pxt.rearrange("k (o p t) -> k o p t", o=1, p=K)
                        .broadcast_to((HID, K, K, NO)),
                    0.0,
                    ets.rearrange("k (q p o) -> k q p o", q=K, o=1)
                        .broadcast_to((HID, K, K, NO)),
                    op0=OP.add, op1=OP.add)

            # diagonal: dmw = u-diag + xr, dmd = dmw + etdelta; scores + exp
            delts = [None] * BL
            for b in range(BL):
                v = V[b]
                dmwd = wp.tile([128, 2 * N], F16, tag="dmwd")
                for q in range(K):
                    nc.vector.scalar_tensor_tensor(
                        dmwd[:, q * NO:(q + 1) * NO],
                        v["ub"][:, q * (N + NO):q * (N + NO) + NO],
                        0.0, v["xrb"][:, q * NO:(q + 1) * NO],
                        op0=OP.add, op1=OP.add)
                nc.vector.tensor_tensor(
                    dmwd[:, N:2 * N].rearrange("k (q t) -> k q t", q=K),
                    dmwd[:, 0:N].rearrange("k (q t) -> k q t", q=K),
                    etds.rearrange("k (q o) -> k q o", o=1)
                        .broadcast_to((HID, K, NO)),
                    op=OP.add)
                ldm = wp.tile([128, 2 * N], F16, tag="ldm")
                nc.scalar.activation(ldm, dmwd, AF.Prelu, bias=zb, alpha=NEG)
                psd = pd.tile([128, 2 * H], F32, tag="psd")
                nc.tensor.matmul(psd[:, 0:H], ldm[:, 0:N], ab16s,
                                 start=True, stop=True, skip_group_check=True)
                nc.tensor.matmul(psd[:, H:2 * H], ldm[:, N:2 * N], ab16s,
                                 start=True, stop=True, skip_group_check=True)
                esd = sp.tile([128, 2 * H], F16, tag="esd")
                nc.scalar.activation(esd, psd, AF.Exp, bias=zb)
                delt = sp.tile([128, H], F16, tag=f"delt{b}")
                nc.vector.tensor_tensor(delt, esd[:, H:2 * H], esd[:, 0:H],
                                        op=OP.subtract)
                delts[b] = delt

            # ---- message blocks over target nodes j, graphs interleaved ----
            psbs = [None] * BL
            for half in range(2):
                for b in range(BL):
                    psbs[b] = ps.tile([128, (N // 2) * H], F32, tag="psb", name=f"psb{b}")
                for blk in range(NBB // 2):
                    j0 = half * (N // 2) + blk * JBB
                    q = j0 // NO
                    path = {2: 2, 5: 2, 7: 4}.get(half * (NBB // 2) + blk, 1)
                    for b in range(BL):
                        v = V[b]
                        psb = psbs[b]
                        if path == 2:
                            # PE builds m into PSUM, lrelu on ScalarE
                            for sub in range(2):
                                j0s = j0 + sub * (JBB // 2)
                                pmb = pm.tile([128, (JBB // 2) * N], F32,
                                              tag="pmb")
                                pm3 = pmb.rearrange("k (j i) -> k j i",
                                                    j=JBB // 2)
                                for g in range(2):
                                    g0 = g * (JBB // 4)
                                    nc.tensor.matmul(
                                        pm3[:, g0:g0 + JBB // 4, :],
                                        ident16[:],
                                        v["ub"][:, q * N:(q + 1) * N]
                                            .rearrange("k (o i) -> k o i", o=1)
                                            .broadcast_to((HID, JBB // 4, N)),
                                        start=True, stop=False,
                                        skip_group_check=True)
                                    nc.tensor.matmul(
                                        pm3[:, g0:g0 + JBB // 4, :],
                                        ident16[:],
                                        v["xrb"][:, j0s + g0:j0s + g0 + JBB // 4]
                                            .rearrange("k (j o) -> k j o", o=1)
                                            .broadcast_to((HID, JBB // 4, N)),
                                        start=False, stop=True,
                                        skip_group_check=True)
                                ma8 = mb.tile([128, (JBB // 2) * N], F16,
                                              tag="ma8")
                                nc.scalar.activation(ma8, pmb, AF.Prelu,
                                                     bias=zb, alpha=NEG)
                                for t in range(JBB // 2):
                                    jl = blk * JBB + sub * (JBB // 2) + t
                                    nc.tensor.matmul(
                                        psb[:, jl * H:(jl + 1) * H],
                                        ma8[:, t * N:(t + 1) * N], ab16s,
                                        start=True, stop=True,
                                        skip_group_check=True)
                        else:
                            mp16 = mb.tile([128, JBB * N], F16, tag="mp")
                            nc.vector.tensor_tensor(
                                mp16.rearrange("k (j i) -> k j i", j=JBB),
                                v["ub"][:, q * N:(q + 1) * N]
                                    .rearrange("k (o i) -> k o i", o=1)
                                    .broadcast_to((HID, JBB, N)),
                                v["xrb"][:, j0:j0 + JBB]
                                    .rearrange("k (j o) -> k j o", o=1)
                                    .broadcast_to((HID, JBB, N)),
                                op=OP.add)
                            ma16 = mb.tile([128, JBB * N], F16, tag="ma")
                            if path == 4:
                                nc.vector.scalar_tensor_tensor(
                                    ma16[:], mp16[:], float(NEG), mp16[:],
                                    op0=OP.mult, op1=OP.max)
                            else:
                                nc.scalar.activation(ma16, mp16, AF.Prelu,
                                                     bias=zb, alpha=NEG)
                            for t in range(JBB):
                                jl = blk * JBB + t
                                nc.tensor.matmul(
                                    psb[:, jl * H:(jl + 1) * H],
                                    ma16[:, t * N:(t + 1) * N], ab16s,
                                    start=True, stop=True,
                                    skip_group_check=True)
                        if blk == NBB // 2 - 1:
                            nc.scalar.activation(
                                v["esb"][:, half * (N // 2) * H:
                                         (half + 1) * (N // 2) * H],
                                psb, AF.Exp, bias=zb)

            # aggregate + normalizer + diagonal correction
            osbs = [None] * BL
            for b in range(BL):
                v = V[b]
                po = pg.tile([128, H * AUG], F32, tag="pg")
                es3 = v["esb"].rearrange("i (j h) -> i j h", h=H)
                for h in range(H):
                    nc.tensor.matmul(
                        po[:, h * AUG:(h + 1) * AUG],
                        es3[:, :, h],
                        v["xlab"][:, h * AUG:(h + 1) * AUG],
                        start=True, stop=True)
                po3 = po.rearrange("j (h q) -> j h q", q=AUG)
                delt = delts[b]
                dtmp = wp.tile([128, H * C], F16, tag="dtmp")
                nc.vector.tensor_tensor(
                    dtmp.rearrange("j (h c) -> j h c", c=C),
                    delt.rearrange("j (h o) -> j h o", o=1)
                        .broadcast_to((128, H, C)),
                    v["xlab"].rearrange("i (h q) -> i h q", q=AUG)[:, :, 0:C],
                    op=OP.mult)
                nc.vector.tensor_add(
                    po3[:, :, 0:C], po3[:, :, 0:C],
                    dtmp.rearrange("j (h c) -> j h c", c=C))
                nc.vector.tensor_add(
                    po3[:, :, 16:17], po3[:, :, 16:17],
                    delt.rearrange("j (h o) -> j h o", o=1))
                zc = sp.tile([128, H], F32, tag="zc")
                nc.vector.tensor_copy(
                    zc.rearrange("j (h o) -> j h o", o=1),
                    po3[:, :, 16:17])
                rz = sp.tile([128, H], F32, tag="rz")
                nc.vector.reciprocal(rz, zc)
                o_sb = wp.tile([128, HID], F32, tag=f"osb{b}")
                nc.vector.tensor_mul(
                    o_sb.rearrange("j (h c) -> j h c", c=C),
                    po3[:, :, 0:C],
                    rz.rearrange("j (h o) -> j h o", o=1).broadcast_to((128, H, C)))
                osbs[b] = o_sb

            # projection + LN + relu + residual
            for b in range(BL):
                v = V[b]
                pto = pt.tile([128, 128], F32, tag="ptr")
                nc.tensor.transpose(pto, osbs[b][:], ident[:])
                oT = wp.tile([128, HID], F16, tag="oT")
                nc.scalar.activation(oT, pto, AF.Identity, bias=zb)
                ppj = pg.tile([128, HID], F32, tag="pg")
                nc.tensor.matmul(ppj, oT[:], pws, start=True, stop=False)
                nc.tensor.matmul(ppj, ones16_r[:], brow_s[:, (6 + l) * HID:(7 + l) * HID], start=False, stop=True)
                lno = wp.tile([128, HID], F32, tag=f"lno{b}")
                _ln_free(nc, wp, sp, ppj[:], lnv(LNG0 + l), lnv(LNB0 + l), lno[:], f"c{b}", zb, epsb)
                rl = wp.tile([128, HID], F32, tag=f"rl{b}")
                nc.scalar.activation(rl, lno, AF.Relu, bias=zb)
                hn = hp.tile([128, HID], F32, tag=f"h{b}")
                nc.vector.tensor_add(hn, rl, h_cur[b])
                h_cur[b] = hn

        # ======== pooling + head ========
        for b in range(BL):
            pa = pg.tile([128, 1], F32, tag="pg")
            nc.tensor.matmul(pa, h_cur[b][:], ones_c[:], start=True, stop=True)
            hagg = sp.tile([128, 1], F32, tag="hagg")
            nc.vector.tensor_copy(hagg, pa)
            pr = pg.tile([1, 1], F32, tag="pg")
            nc.tensor.matmul(pr, hagg[:], ow_s[:], start=True, stop=True)
            res = sp.tile([1, 1], F32, tag="res")
            nc.scalar.activation(res, pr, AF.Identity, bias=ob_s[0:1, 0:1])
            nc.sync.dma_start(out[b:b + 1, :], res[:])

    nc.compile()
    return nc


def pack_inputs(inputs):
    """Full model inputs -> per-core in_maps (host-side shard + re-layout)."""
    f = {k: np.asarray(v, dtype=np.float32) if k != "cat" else np.asarray(v)
         for k, v in inputs.items()}

    # the kernel exploits the orbit structure of cat; verify it holds
    cat = np.asarray(f["cat"], dtype=np.int64)
    pos_ = np.arange(N) // NO
    i_, j_ = np.arange(N)[:, None], np.arange(N)[None, :]
    cat_exp = np.where(i_ == j_, K * K + pos_[:, None],
                       pos_[:, None] * K + pos_[None, :])
    assert np.array_equal(cat, cat_exp), "cat does not match orbit structure"

    att = f["att"]
    abk = np.zeros((HID, L * H), np.float32)
    for l in range(L):
        for h in range(H):
            abk[h * C:(h + 1) * C, l * H + h] = att[l, h]

    pb_eff = np.stack([f["cb"][l] @ f["pW"][l] + f["pb"][l] for l in range(L)])

    # edge-category transforms: e20[l] = emb @ We[l]  -> [20, HID]
    # off-diag cat(p source, q target) = p*K+q; diag cat = K*K+q
    et = np.zeros((HID, L * K * K), np.float16)
    etd = np.zeros((HID, L * K), np.float16)
    for l in range(L):
        e20 = f["emb"] @ f["We"][l]                 # [20, HID]
        for q in range(K):
            for p in range(K):
                et[:, l * K * K + q * K + p] = e20[p * K + q]
            etd[:, l * K + q] = e20[K * K + q] - e20[q * K + q]

    lnvecs = [f["ln1_g"], f["ln1_b"], f["ln2_g"], f["ln2_b"],
              *[f["lng"][l] for l in range(L)], *[f["lnb"][l] for l in range(L)]]
    lnr = np.ascontiguousarray(
        np.broadcast_to(np.concatenate(lnvecs)[None, :], (HID, 12 * HID)))

    def stackw(w):  # [L, k, hc] -> [k, L*hc] so sbuf slice l is W[l][k, hc]
        return np.ascontiguousarray(
            w.transpose(1, 0, 2).reshape(HID, L * HID)).astype(np.float16)

    shared = {
        "wl": stackw(f["Wl"]), "wr": stackw(f["Wr"]), "pw": stackw(f["pW"]),
        "et": et, "etd": etd, "ab16": abk.astype(np.float16),
        "brow": np.concatenate([f["mlp_b1"], f["mlp_b2"],
                                f["bl"].ravel(), pb_eff.ravel()])
            .reshape(1, 10 * HID).astype(np.float16),
        "mw1": f["mlp_w1"], "mw2": f["mlp_w2"].astype(np.float16),
        "lnr": lnr.astype(np.float16), "ow": f["out_w"].reshape(HID, 1),
        "ob": f["out_b"].reshape(1, 1),
    }
    in_maps = []
    for c in range(NCORES):
        xTc = np.ascontiguousarray(
            f["x"][c * BL:(c + 1) * BL].transpose(2, 0, 1)).reshape(2, BL * N)
        m = dict(shared)
        m["xT"] = xTc
        in_maps.append(m)
    return in_maps


_NC = None
LAST_EXEC_NS = None


def kernel(**inputs) -> np.ndarray:
    global _NC, LAST_EXEC_NS
    from concourse.bass_utils import run_bass_kernel_spmd
    if _NC is None:
        _NC = build_nc()
    import os
    in_maps = pack_inputs(inputs)
    trace = bool(os.environ.get("KERNEL_TRACE"))
    r = run_bass_kernel_spmd(_NC, in_maps, core_ids=list(range(NCORES)),
                             trace=trace)
    LAST_EXEC_NS = r.exec_time_ns
    out = np.concatenate([r.results[c]["out"] for c in range(NCORES)], axis=0)
    return out.astype(np.float32)
